# revision 27
# baseline (speedup 1.0000x reference)
"""Trainium2 Bass kernel for nn_AdaptiveModelV3 (LSTM + episodic memory read).

Sharding (hardcoded per spec): data-parallel over batch on 8 NeuronCores
(512 batch each); the tiny memory bank (200x32 keys/vals from sample 0)
is computed host-side and replicated.  No collectives.

Per-core device kernel (bf16 compute, fp32 PSUM accumulation):
  - LSTM scan over 201 steps in gate-transposed layout (partitions =
    128 hidden units of a gate, free dim = batch) so h_t leaves the
    pointwise stage already transposed for the next step's matmul.
  - Input projection at K=64 with TWO gates computed concurrently via
    PE row tiling (tile_position bands at rows 0-63/64-127; x data
    duplicated in both partition bands) -- ic is nearly free.  Falls
    back to K=65 with a bias ones-row if LSTM biases are nonzero.
  - Gate order [i, f, o, g] with the g-gate weights pre-scaled by 2:
    tanh(x) = 2*sigmoid(2x) - 1, so ONE merged Sigmoid covers all four
    gates per stream per step; tanh(c) is a direct ACT op.
  - Batch split into 2 streams of 256, phase-interleaved per engine so
    one stream's serial chain (mm -> sigmoid -> DVE -> tanh -> h -> mm)
    hides under the other's work on ACT/DVE/PE.
  - PSUM: one bank per gate per stream ([128, 4, 512] half-used,
    bufs=1).  start=True clears has_written at PSUM-BANK granularity,
    so bank isolation is what allows the input-projection matmuls to
    prefetch ahead of the h-dependent recurrent matmuls in any order.

Head/tail schedule (the steady state is latency-chain-bound at
~3.94us/step; the recoverable time was the ~49us head + 12us tail of
the previous revision):
  - ALL constants ride in TWO packed DMAs at the HEAD of the sync/
    HWDGE queue (before the 26MB x-chunk stream), so they no longer
    serialize against each other, lose DMA-semaphore slots, or starve
    behind the x-chunk bandwidth.  Chunk 0 is a single step so step
    0's input projection is gated by a 131KB transfer.
  - The sigmoid ACT table and the custom-DVE uop program are pre-warmed
    at boot with tiny dummy ops (the auto-inserted table load otherwise
    sits BEHIND sigma_0's semaphore wait; the first full-size custom op
    otherwise pays ~5us on step 0's chain).  A short t=0 filler burst
    ramps the HAM clock without delaying sigma_A(0) -- kept short so
    the two streams' steady phase (~1us) locks in correctly.
  - Attention q / ||q||^2 matmuls run pre-scan in bf16 (one psum bank,
    freed via SBUF copies before the scan claims all 8 banks); the
    rsqrt Newton chain runs all-bf16 on the DVE, emitted right after
    step 0 where the DVE idles during the clock ramp.  (Pool/Q7 tensor
    ops were observed to stall concurrent custom-DVE ops -- keep the
    Pool engine compute-free.)
  - Logits/exp/softmax/ctx/output head all run AFTER the scan: the exp
    ACT-table load (the set switch costs ~2.7us and evicts sigmoid)
    lands in the tail, and the woh@h matmuls fire the moment the final
    h is ready.  Output bias rides as a ones-row in the woc matmul.
"""

import sys

import numpy as np

try:
    import concourse.bass as bass
except ImportError:  # pragma: no cover
    sys.path.insert(0, "/opt/trn_rl_repo")
    import concourse.bass as bass

import ml_dtypes
from contextlib import ExitStack

import concourse.mybir as mybir
import concourse.tile as tile
from concourse import bacc
from concourse.bass_utils import run_bass_kernel_spmd

BF16 = ml_dtypes.bfloat16

# Problem constants (hardcoded per spec).
B, S, DIN = 4096, 201, 64
H = 128
KD, DK, DV = 32, 32, 32
DOUT = 32
TEMP = 0.1
NOVELTY = 0.5
EPS = 1e-8
NSLOTS = S - 1  # 200

NCORES = 8
BC = B // NCORES  # 512 batch per core
NSTREAM = 2
BS = BC // NSTREAM  # 256 batch per stream
# time chunks (sum = 201): 1-step chunk 0 so step 0's input projection is
# gated by a 131KB transfer, not a multi-MB one; small max -> small xpool
CHUNKS = [1, 9, 48, 48, 48, 47]
CHUNK_START = [0, 1, 10, 58, 106, 154]

# Gate order in reference (PyTorch): [i, f, g, o].  Our PSUM layout is
# [i, f, o, g] so sigmoid covers a contiguous [i,f,o] block.
GATE_SRC = [0, 1, 3, 2]
D1_FILL = 12   # PE filler mms after rc (real-work duty alone lets HAM throttle)
F32 = mybir.dt.float32

# --- custom DVE op: out = Src0*(Src1*s0 + s1) ------------------------------
# Fuses the LSTM's tg = 2*sigma(2g)-1 fixup into the i*g product (one DVE
# instruction instead of tensor_scalar + tensor_tensor on the critical
# sigmoid->c->tanh chain).  Registered at import into dve_ops' tables; the
# per-NEFF DVE table mechanism ships the uop program, no firmware change.
from concourse.dve_spec import Spec as _Spec, Src0 as _Src0, Src1 as _Src1
from concourse.dve_spec import C0 as _C0, C1 as _C1, lower as _dve_lower
from concourse.dve_uop import DveOpSpec as _DveOpSpec
from concourse import dve_ops as _DO


def _register_tgii():
    if "LSTM_TGII" in _DO._SUB_OPCODE_FOR_NAME:
        return next(o for o in _DO.OPS if o.name == "LSTM_TGII")
    op = _DO.DveOp(
        "LSTM_TGII", _Spec(body=_Src0 * (_Src1 * _C0 + _C1)),
        subdim=False, uops_sha={},
    )
    _DO.OPS.append(op)
    opcode = 1 + len(_DO.OPS) - 1
    _DO._SUB_OPCODE_FOR_NAME["LSTM_TGII"] = opcode
    _DO.CUSTOM_DVE_SPECS["LSTM_TGII"] = op.spec
    for ver in ("v3", "v4"):
        try:
            _DO._COMPILE_CACHE[("LSTM_TGII", ver)] = _DveOpSpec(
                name="LSTM_TGII", opcode=opcode,
                uops=_dve_lower(op.spec, ver=ver), rd1_en=True,
            )
        except Exception:
            pass
    return op


TGII = _register_tgii()
BF = mybir.dt.bfloat16
AF = mybir.ActivationFunctionType
ALU = mybir.AluOpType


def _pack_layout(use_k64):
    """Column offsets of each constant inside the packed bf16 blob."""
    wih_cols = 2 * H if use_k64 else 4 * H
    o = {}
    o["wih"] = 0
    o["whh"] = o["wih"] + wih_cols
    o["xq"] = o["whh"] + 4 * H
    o["wk"] = o["xq"] + BC
    o["knt"] = o["wk"] + DK
    o["v1"] = o["knt"] + NSLOTS
    o["v2"] = o["v1"] + DV
    o["woh"] = o["v2"] + DV
    o["wocb"] = o["woh"] + DOUT
    o["_end"] = o["wocb"] + DOUT
    return o


def build_kernel(s_steps=S, no_rc=False, use_k64=True):
    nc = bacc.Bacc()

    XP = 2 * DIN if use_k64 else DIN + 1  # k64: x duplicated in both row bands
    WIH_ROWS = 2 * DIN if use_k64 else DIN + 1
    NPAIR = 2 if use_k64 else 4
    O = _pack_layout(use_k64)
    CB = O["_end"]

    xaug = nc.dram_tensor("xaug", [XP, S, BC], BF, kind="ExternalInput")
    cpak = nc.dram_tensor("cpak", [128, CB], BF, kind="ExternalInput")
    fpak = nc.dram_tensor("fpak", [128, 2], F32, kind="ExternalInput")
    out = nc.dram_tensor("out", [DOUT, BC], F32, kind="ExternalOutput")

    with tile.TileContext(nc) as tc, ExitStack() as ctx:
        consts = ctx.enter_context(tc.tile_pool(name="consts", bufs=1))
        xpool = ctx.enter_context(tc.tile_pool(name="xp", bufs=2))
        work = ctx.enter_context(tc.tile_pool(name="work", bufs=3))

        # ---- ALL DMAs ride the sync/HWDGE queue in priority order: the
        # two packed const blobs (needed by everything) FIRST, then the x
        # chunks.  Sequential per-queue transfer order = explicit priority;
        # nothing starves behind the 26MB x stream.
        sb_c = consts.tile([128, CB], BF, tag="cpak")
        nc.sync.dma_start(sb_c[:], cpak[:])
        sb_f = consts.tile([128, 2], F32, tag="fpak")
        nc.sync.dma_start(sb_f[:], fpak[:])
        xc_tiles = {}
        for ci, tc_len in enumerate(CHUNKS):
            t0c = CHUNK_START[ci]
            xc = xpool.tile([XP, max(CHUNKS), BC], BF, tag="xc", name=f"xc{ci}")
            nc.sync.dma_start(xc[:, :tc_len], xaug[:, t0c : t0c + tc_len, :])
            xc_tiles[ci] = xc

        def c_sl(rows, off, width):
            return sb_c[0:rows, off : off + width]

        # ---- small memset consts (DVE, ~100ns each, t~0)
        ones_q = consts.tile([DK, DK], BF, tag="ones_q")
        nc.vector.memset(ones_q[:], 1.0)
        ones_s1 = consts.tile([128, DK], BF, tag="ones_s1")
        nc.vector.memset(ones_s1[:], 1.0)
        ones_s2 = consts.tile([NSLOTS - 128, DK], BF, tag="ones_s2")
        nc.vector.memset(ones_s2[:], 1.0)
        ctxb1 = consts.tile([DV + 1, BC], BF, tag="ctxb1")
        nc.vector.memset(ctxb1[DV : DV + 1, :], 1.0)
        # pre-warm the custom-DVE uop path at FULL op shape: the first
        # 256-col custom op otherwise pays ~4.9us on step 0's chain.
        tgw_in = consts.tile([128, BS], BF, tag="tgw_in")
        nc.vector.memset(tgw_in[:], 0.5)
        tg_warm = consts.tile([128, BS], BF, tag="tg_warm")
        nc.vector._custom_dve(
            TGII, out=tg_warm[:], in0=tgw_in[:], in1=tgw_in[:],
            s0=2.0, s1=-1.0,
        )
        # pre-warm the sigmoid ACT table: the auto-inserted table load
        # sits BEHIND sigma_0's semaphore wait on the ACT queue, so
        # without this it only starts once step 0's matmuls finish.
        sig_warm = consts.tile([128, 8], BF, tag="sig_warm")
        nc.scalar.activation(sig_warm[:], ones_s1[:, 0:8], AF.Sigmoid)

        # (no PE warm-up burst: the scan's own filler duty ramps the HAM
        # clock within the first couple of steps; a pre-scan burst either
        # blocks a psum bank or delays step 0 more than the ramp costs)

        # ---- attention q / ||q||^2: pre-scan, one psum bank, results
        # copied straight to SBUF so the bank frees before the scan.
        # All bf16 (the downstream logits matmul is bf16 anyway).
        # NOTE: no Pool-engine compute anywhere -- Q7 tensor ops were
        # observed to stall concurrent custom-DVE ops by multiple us.
        psA_cm = tc.tile_pool(name="psA", bufs=1, space="PSUM")
        psA = psA_cm.__enter__()
        q_ps = psA.tile([DK, BC], F32, tag="q")
        nc.tensor.matmul(
            q_ps[:], c_sl(KD + 1, O["wk"], DK), c_sl(KD + 1, O["xq"], BC),
            start=True, stop=True,
        )
        qc = consts.tile([DK, BC], BF, tag="qc")
        nc.vector.tensor_copy(qc[:], q_ps[:])
        q2 = consts.tile([DK, BC], BF, tag="q2")
        nc.vector.tensor_tensor(q2[:], qc[:], qc[:], ALU.mult)
        nc.tensor.matmul(q_ps[:], ones_q[:], q2[:], start=True, stop=True)
        nsb = consts.tile([DK, BC], BF, tag="nsb")
        nc.vector.tensor_copy(nsb[:], q_ps[:])
        psA_cm.__exit__(None, None, None)

        def emit_newton():
            # rsqrt(n) Newton on the DVE, all-bf16 (2x/4x modes): clamped
            # linear seed + 2 iters y <- y*(1.5 - 0.5*n*y^2).  Emitted
            # after step 0's DVE ops: the DVE is idle there while the
            # ACT/PE ramp, so the ~4us chain hides in the early bubbles.
            y0 = consts.tile([DK, BC], BF, tag="y0")
            nc.vector.tensor_scalar(y0[:], nsb[:], -0.002953, 0.2917,
                                    ALU.mult, ALU.add)
            yc = consts.tile([DK, BC], BF, tag="yc")
            nc.vector.tensor_scalar(yc[:], y0[:], 0.085, 0.40, ALU.max, ALU.min)
            s32 = yc
            for it in range(2):
                t2 = consts.tile([DK, BC], BF, tag=f"nw_t{it}")
                nc.vector.tensor_tensor(t2[:], s32[:], s32[:], ALU.mult)
                u2 = consts.tile([DK, BC], BF, tag=f"nw_u{it}")
                nc.vector.tensor_tensor(u2[:], t2[:], nsb[:], ALU.mult)
                v2 = consts.tile([DK, BC], BF, tag=f"nw_v{it}")
                nc.vector.tensor_scalar(v2[:], u2[:], -0.5, 1.5,
                                        ALU.mult, ALU.add)
                y2 = consts.tile([DK, BC], BF, tag=f"nw_y{it}")
                nc.vector.tensor_tensor(y2[:], s32[:], v2[:], ALU.mult)
                s32 = y2
            qn = consts.tile([DK, BC], BF, tag="qn")
            nc.vector.tensor_tensor(qn[:], qc[:], s32[:], ALU.mult)
            return qn

        # ========================= LSTM scan ===========================
        psL_cm = tc.tile_pool(name="psL", bufs=1, space="PSUM")
        psL = psL_cm.__enter__()
        h_tiles = [None, None]
        c_tiles = [None, None]

        _dummy_rhs = sb_c[:, O["whh"] + H : O["whh"] + 3 * H]
        _dummy_lhs = sb_c[:, O["whh"] + H : O["whh"] + H + 32]
        # psum per stream: [128, 4, 512] fp32, ONE BANK PER GATE (cols
        # 0:BS used).  start=True clears has_written at bank granularity,
        # so bank isolation lets ic mms prefetch in any order before the
        # h-dependent rc mms.  bufs=1: 4 banks x 2 streams = all of PSUM;
        # the only psum reader is the merged sigmoid, so step t+1's ic
        # mms just wait on sigma(t).
        chunk_of = []
        for ci, tc_len in enumerate(CHUNKS):
            chunk_of += [(ci, t) for t in range(tc_len)]
        for t in range(s_steps):
            xc = xc_tiles[chunk_of[t][0]]
            ti = chunk_of[t][1]
            ps_t = []
            for s in range(NSTREAM):
                ps = psL.tile([128, 4, 512], F32, tag=f"lstm{s}")
                ps_t.append(ps)
            # input-projection mms: prefetchable (no h dependency).
            # k64: two gates run CONCURRENTLY in the PE array via row
            # tiling (bands at rows 0-63 / 64-127); x is duplicated in
            # both partition bands so each band streams its own copy.
            for p in range(NPAIR):
                for s in range(NSTREAM):
                    if use_k64:
                        for b in range(2):
                            g = 2 * p + b
                            lh = sb_c[
                                64 * b : 64 * (b + 1),
                                O["wih"] + p * H : O["wih"] + (p + 1) * H,
                            ]
                            rh = xc[64 * b : 64 * (b + 1), ti,
                                    s * BS : (s + 1) * BS]
                            nc.tensor.matmul(
                                ps_t[s][:, g, 0:BS], lh, rh,
                                start=True, stop=(t == 0),
                                tile_position=(64 * b, 0),
                            )
                    else:
                        xs = xc[:, ti, s * BS : (s + 1) * BS]
                        lh = sb_c[
                            0:WIH_ROWS,
                            O["wih"] + p * H : O["wih"] + (p + 1) * H,
                        ]
                        nc.tensor.matmul(
                            ps_t[s][:, p, 0:BS], lh, xs,
                            start=True, stop=(t == 0),
                        )

            # PE filler: keep duty high so HAM holds the 2.4GHz clock.
            # 32-col weights make the LDWEIGHTS ~27ns (vs 96 for 128-col)
            # and the long 256-col stream supplies the array duty; dummies
            # write the unused scratch half of the step's psum banks.
            def dummy(n, tile_idx=0, gate=0):
                for _ in range(n):
                    nc.tensor.matmul(
                        ps_t[tile_idx][0:32, gate, BS : BS + 256],
                        _dummy_lhs,
                        _dummy_rhs[:, 0:256], start=False, stop=False,
                        skip_group_check=True,
                    )
            if t == 0:
                # HAM p-state ramp burst right after step-0's input
                # projections.  Target = a stream-1 gate bank that does
                # NOT overlap the attention q bank (stream 0, gate 0), so
                # sigma_A(0) doesn't wait on it; kept short so sigma_B(0)
                # stays ~1.1us behind sigma_A(0) -- the steady-state
                # stream phase locks in from these initial conditions.
                dummy(12, tile_idx=1, gate=2)
            # recurrent mms
            if t > 0 and not no_rc:
                for s in range(NSTREAM):
                    for g in range(4):
                        nc.tensor.matmul(
                            ps_t[s][:, g, 0:BS],
                            sb_c[:, O["whh"] + g * H : O["whh"] + (g + 1) * H],
                            h_tiles[s][:],
                            start=False, stop=True,
                        )

            # merged sigmoid over all 4 gates (g gate pre-scaled by 2 so
            # tanh(x) = 2*sigmoid(2x)-1 costs only a DVE fixup)
            sigs = []
            for s in range(NSTREAM):
                sig = work.tile([128, 4, BS], BF, tag=f"sig{s}")
                nc.scalar.activation(sig[:], ps_t[s][:, :, 0:BS], AF.Sigmoid)
                sigs.append(sig)
            if t > 0:
                # PE warmth filler AFTER sigma_A's emission: the bank-order
                # dep (scratch shares stream A's banks) is then on the
                # already-completed sigma_A, so the fillers run in the PE's
                # idle window without delaying sigma, ic or rc.
                dummy(D1_FILL, 0)
            iis, ffs = [], []
            for s in range(NSTREAM):
                sig = sigs[s]
                # fused ii = sigma_i * (2*sigma(2g) - 1) -- one DVE instr on
                # the critical chain instead of tensor_scalar+tensor_tensor
                ii = work.tile([128, BS], BF, tag=f"ii{s}")
                nc.vector._custom_dve(
                    TGII, out=ii[:], in0=sig[:, 0], in1=sig[:, 3],
                    s0=2.0, s1=-1.0,
                )
                iis.append(ii)
                if t > 0:
                    ff = work.tile([128, BS], BF, tag=f"ff{s}")
                    nc.vector.tensor_tensor(ff[:], sig[:, 1], c_tiles[s][:], ALU.mult)
                    ffs.append(ff)
            c_news = []
            for s in range(NSTREAM):
                c_new = work.tile([128, BS], BF, tag=f"c{s}")
                if t > 0:
                    nc.vector.tensor_tensor(c_new[:], iis[s][:], ffs[s][:], ALU.add)
                else:
                    nc.vector.tensor_copy(c_new[:], iis[s][:])
                c_news.append(c_new)
            c_tiles = c_news
            h_news = []
            for s in range(NSTREAM):
                tcc = work.tile([128, BS], BF, tag=f"tc{s}")
                nc.scalar.activation(tcc[:], c_tiles[s][:], AF.Tanh)
                h_new = work.tile([128, BS], BF, tag=f"h{s}")
                nc.vector.tensor_tensor(h_new[:], sigs[s][:, 2], tcc[:], ALU.mult)
                h_news.append(h_new)
            h_tiles = h_news
            if t == 0:
                qn = emit_newton()

        psL_cm.__exit__(None, None, None)

        # ================ attention read + output head (tail) ==========
        # Emitted after the scan on every engine queue: the exp table
        # load (which evicts the sigmoid set) lands here, and l/exp/
        # softmax deps (qn from the Pool Newton) are long since ready.
        psH_cm = tc.tile_pool(name="psH", bufs=1, space="PSUM")
        psH = psH_cm.__enter__()
        # one psum bank PER STREAM: a second start=True in a shared bank
        # would clear the first stream's has_written state (bank-granular)
        out_ps = [
            psH.tile([DOUT, BS], F32, tag=f"o{s}", name=f"out_ps{s}")
            for s in range(NSTREAM)
        ]
        for s in range(NSTREAM):
            nc.tensor.matmul(
                out_ps[s][:], c_sl(H, O["woh"], DOUT), h_tiles[s][:],
                start=True, stop=False,
            )
        # logits chunks: L = kn @ qn  ([slots, BC]); bf16 -> 1 cyc/row on PE
        l1_ps = psH.tile([128, BC], F32, tag="l1")
        nc.tensor.matmul(
            l1_ps[:], c_sl(DK, O["knt"], 128), qn[:], start=True, stop=True
        )
        l2_ps = psH.tile([NSLOTS - 128, BC], F32, tag="l2")
        nc.tensor.matmul(
            l2_ps[:], sb_c[0:DK, O["knt"] + 128 : O["knt"] + NSLOTS], qn[:],
            start=True, stop=True,
        )
        # softmax without max-subtraction: masked slots get bias -1e5 ->
        # exp underflows to 0.  bias_ptr = per-partition mask column.
        e1 = consts.tile([128, BC], BF, tag="e1")
        nc.scalar.activation(e1[:], l1_ps[:], AF.Exp, bias=sb_f[0:128, 0:1],
                             scale=1.0 / TEMP)
        e2 = consts.tile([NSLOTS - 128, BC], BF, tag="e2")
        nc.scalar.activation(e2[:], l2_ps[:], AF.Exp,
                             bias=sb_f[0 : NSLOTS - 128, 1:2],
                             scale=1.0 / TEMP)
        # denominator, replicated over DK partitions; rs = 1/S
        s_ps = psH.tile([DK, BC], F32, tag="s")
        nc.tensor.matmul(s_ps[:], ones_s1[:], e1[:], start=True, stop=False)
        nc.tensor.matmul(s_ps[:], ones_s2[:], e2[:], start=False, stop=True)
        rs = consts.tile([DK, BC], F32, tag="rs")
        nc.vector.reciprocal_approx_fast(rs[:], s_ps[:])
        # ctxT = V^T @ E -> [DV, BC]; normalize into ctxb1 rows 0:DV
        # (row DV is the ones-row that carries b_o through the matmul)
        cx_ps = psH.tile([DV, BC], F32, tag="c")
        nc.tensor.matmul(cx_ps[:], c_sl(128, O["v1"], DV), e1[:],
                         start=True, stop=False)
        nc.tensor.matmul(cx_ps[:], sb_c[0 : NSLOTS - 128, O["v2"] : O["v2"] + DV],
                         e2[:], start=False, stop=True)
        nc.vector.tensor_tensor(ctxb1[0:DV, :], cx_ps[:], rs[:], ALU.mult)
        for s in range(NSTREAM):
            cols = slice(s * BS, (s + 1) * BS)
            nc.tensor.matmul(
                out_ps[s][:], c_sl(DV + 1, O["wocb"], DOUT),
                ctxb1[:, cols], start=False, stop=True,
            )
        out_sb = consts.tile([DOUT, BC], F32, tag="out_sb")
        for s in range(NSTREAM):
            cols = slice(s * BS, (s + 1) * BS)
            nc.vector.tensor_copy(out_sb[:, cols], out_ps[s][:])
        nc.sync.dma_start(out[:], out_sb[:])
        psH_cm.__exit__(None, None, None)

    nc.finalize()
    return nc


def _prep_inputs(inputs, W_ih, W_hh, b_ih, b_hh, W_k, b_k, W_o, b_o):
    """Host-side prep: weight layouts, memory bank, per-core shards."""
    f32 = np.float32
    inputs = np.asarray(inputs, f32)
    W_ih = np.asarray(W_ih, f32)
    W_hh = np.asarray(W_hh, f32)
    b = np.asarray(b_ih, f32) + np.asarray(b_hh, f32)
    W_k = np.asarray(W_k, f32)
    b_k = np.asarray(b_k, f32)
    W_o = np.asarray(W_o, f32)
    b_o = np.asarray(b_o, f32)

    use_k64 = bool(np.all(b == 0.0))
    O = _pack_layout(use_k64)
    CB = O["_end"]

    # LSTM weights, gate-transposed, gate order [i,f,o,g]; tanh(x) =
    # 2*sigmoid(2x)-1 folded into the g gate (position 3).
    whh = np.zeros((H, 4, H), f32)
    for j, gs in enumerate(GATE_SRC):
        rows = slice(gs * H, (gs + 1) * H)
        whh[:, j, :] = W_hh[rows].T
    whh[:, 3, :] *= 2.0
    if use_k64:
        # row-tiled pairs: pair p holds gates (2p, 2p+1) in partition
        # bands 0:64 / 64:128
        wih = np.zeros((2 * DIN, 2, H), f32)
        for j, gs in enumerate(GATE_SRC):
            rows = slice(gs * H, (gs + 1) * H)
            band, pair = (j % 2) * DIN, j // 2
            wih[band : band + DIN, pair, :] = W_ih[rows].T
            if j == 3:
                wih[band : band + DIN, pair, :] *= 2.0
        wih_rows, wih_cols = 2 * DIN, 2 * H
    else:
        wih = np.zeros((DIN + 1, 4, H), f32)
        for j, gs in enumerate(GATE_SRC):
            rows = slice(gs * H, (gs + 1) * H)
            wih[:DIN, j, :] = W_ih[rows].T
            wih[DIN, j, :] = b[rows]
        wih[:, 3, :] *= 2.0
        wih_rows, wih_cols = DIN + 1, 4 * H

    # memory bank from sample 0 (host-side, replicated)
    support = inputs[0, :NSLOTS]
    kp, vp = support[:, :KD], support[:, KD:]
    active = vp.sum(axis=-1) >= NOVELTY
    sk = kp @ W_k.T + b_k
    kn = sk / (np.linalg.norm(sk, axis=-1, keepdims=True) + EPS)
    knt = np.ascontiguousarray(kn.T)  # [DK, NSLOTS]
    maskb = np.where(active, 0.0, -1e5).astype(f32)

    wk = np.zeros((KD + 1, DK), f32)
    wk[:KD] = W_k.T
    wk[KD] = b_k

    woh = np.ascontiguousarray(W_o[:, :H].T)        # [H, DOUT]
    wocb = np.zeros((DV + 1, DOUT), f32)            # [woc ; b_o]
    wocb[:DV] = W_o[:, H:].T
    wocb[DV] = b_o

    # ---- packed const blobs
    cpak = np.zeros((128, CB), f32)
    cpak[0:wih_rows, O["wih"] : O["wih"] + wih_cols] = wih.reshape(
        wih_rows, wih_cols
    )
    cpak[0:H, O["whh"] : O["whh"] + 4 * H] = whh.reshape(H, 4 * H)
    cpak[0 : KD + 1, O["wk"] : O["wk"] + DK] = wk
    cpak[0:DK, O["knt"] : O["knt"] + NSLOTS] = knt
    cpak[0:128, O["v1"] : O["v1"] + DV] = vp[0:128]
    cpak[0 : NSLOTS - 128, O["v2"] : O["v2"] + DV] = vp[128:NSLOTS]
    cpak[0:H, O["woh"] : O["woh"] + DOUT] = woh
    cpak[0 : DV + 1, O["wocb"] : O["wocb"] + DOUT] = wocb

    fpak = np.zeros((128, 2), f32)
    fpak[0:128, 0] = maskb[0:128]
    fpak[0 : NSLOTS - 128, 1] = maskb[128:NSLOTS]

    in_maps = []
    for c in range(NCORES):
        shard = inputs[c * BC : (c + 1) * BC]  # [BC, S, DIN]
        xt = shard.transpose(2, 1, 0)  # [DIN, S, BC]
        if use_k64:
            xaug = np.empty((2 * DIN, S, BC), f32)
            xaug[:DIN] = xt
            xaug[DIN:] = xt
        else:
            xaug = np.empty((DIN + 1, S, BC), f32)
            xaug[:DIN] = xt
            xaug[DIN] = 1.0
        cp = cpak.copy()
        cp[0 : KD + 1, O["xq"] : O["xq"] + BC] = np.concatenate(
            [shard[:, S - 1, :KD].T, np.ones((1, BC), f32)], axis=0
        )
        m = dict(
            xaug=xaug.astype(BF16),
            cpak=cp.astype(BF16),
            fpak=fpak,
        )
        in_maps.append(m)
    return in_maps


_CACHED_NC = None


def kernel(inputs, W_ih, W_hh, b_ih, b_hh, W_k, b_k, W_o, b_o,
           _trace=False, _return_raw=False):
    global _CACHED_NC
    in_maps = _prep_inputs(inputs, W_ih, W_hh, b_ih, b_hh, W_k, b_k, W_o, b_o)
    use_k64 = in_maps[0]["xaug"].shape[0] == 2 * DIN
    if _CACHED_NC is None:
        _CACHED_NC = build_kernel(use_k64=use_k64)
    res = run_bass_kernel_spmd(
        _CACHED_NC, in_maps, core_ids=list(range(NCORES)), trace=_trace
    )
    outs = [np.asarray(res.results[i]["out"], np.float32).T for i in range(NCORES)]
    full = np.concatenate(outs, axis=0)
    if _return_raw:
        return full, res
    return full


# revision 29
# speedup vs baseline: 1.0006x; 1.0006x over previous
"""Trainium2 Bass kernel for nn_AdaptiveModelV3 (LSTM + episodic memory read).

Sharding (hardcoded per spec): data-parallel over batch on 8 NeuronCores
(512 batch each); the tiny memory bank (200x32 keys/vals from sample 0)
is computed host-side and replicated.  No collectives.

Per-core device kernel (bf16 compute, fp32 PSUM accumulation):
  - LSTM scan over 201 steps in gate-transposed layout (partitions =
    128 hidden units of a gate, free dim = batch) so h_t leaves the
    pointwise stage already transposed for the next step's matmul.
  - Input projection at K=64 with TWO gates computed concurrently via
    PE row tiling (tile_position bands at rows 0-63/64-127; x data
    duplicated in both partition bands) -- ic is nearly free.  Falls
    back to K=65 with a bias ones-row if LSTM biases are nonzero.
  - Gate order [i, f, o, g] with the g-gate weights pre-scaled by 2:
    tanh(x) = 2*sigmoid(2x) - 1, so ONE merged Sigmoid covers all four
    gates per stream per step; tanh(c) is a direct ACT op.
  - Batch split into 2 streams of 256, phase-interleaved per engine so
    one stream's serial chain (mm -> sigmoid -> DVE -> tanh -> h -> mm)
    hides under the other's work on ACT/DVE/PE.
  - PSUM: one bank per gate per stream ([128, 4, 512] half-used,
    bufs=1).  start=True clears has_written at PSUM-BANK granularity,
    so bank isolation is what allows the input-projection matmuls to
    prefetch ahead of the h-dependent recurrent matmuls in any order.

Head/tail schedule (the steady state is latency-chain-bound at
~3.94us/step; the recoverable time was the ~49us head + 12us tail of
the previous revision):
  - ALL constants ride in TWO packed DMAs at the HEAD of the sync/
    HWDGE queue (before the 26MB x-chunk stream), so they no longer
    serialize against each other, lose DMA-semaphore slots, or starve
    behind the x-chunk bandwidth.  Chunk 0 is a single step so step
    0's input projection is gated by a 131KB transfer.
  - The sigmoid ACT table and the custom-DVE uop program are pre-warmed
    at boot with tiny dummy ops (the auto-inserted table load otherwise
    sits BEHIND sigma_0's semaphore wait; the first full-size custom op
    otherwise pays ~5us on step 0's chain).  A short t=0 filler burst
    ramps the HAM clock without delaying sigma_A(0) -- kept short so
    the two streams' steady phase (~1us) locks in correctly.
  - Attention q / ||q||^2 matmuls run pre-scan in bf16 (one psum bank,
    freed via SBUF copies before the scan claims all 8 banks); the
    rsqrt Newton chain runs all-bf16 on the DVE, emitted right after
    step 0 where the DVE idles during the clock ramp.  (Pool/Q7 tensor
    ops were observed to stall concurrent custom-DVE ops -- keep the
    Pool engine compute-free.)
  - Logits/exp/softmax/ctx/output head all run AFTER the scan: the exp
    ACT-table load (the set switch costs ~2.7us and evicts sigmoid)
    lands in the tail, and the woh@h matmuls fire the moment the final
    h is ready.  Output bias rides as a ones-row in the woc matmul.
"""

import sys

import numpy as np

try:
    import concourse.bass as bass
except ImportError:  # pragma: no cover
    sys.path.insert(0, "/opt/trn_rl_repo")
    import concourse.bass as bass

import ml_dtypes
from contextlib import ExitStack

import concourse.mybir as mybir
import concourse.tile as tile
from concourse import bacc
from concourse.bass_utils import run_bass_kernel_spmd

BF16 = ml_dtypes.bfloat16

# Problem constants (hardcoded per spec).
B, S, DIN = 4096, 201, 64
H = 128
KD, DK, DV = 32, 32, 32
DOUT = 32
TEMP = 0.1
NOVELTY = 0.5
EPS = 1e-8
NSLOTS = S - 1  # 200

NCORES = 8
BC = B // NCORES  # 512 batch per core
NSTREAM = 2
BS = BC // NSTREAM  # 256 batch per stream
# time chunks (sum = 201): 1-step chunk 0 so step 0's input projection is
# gated by a 131KB transfer, not a multi-MB one; small max -> small xpool
CHUNKS = [1, 9, 48, 48, 48, 47]
CHUNK_START = [0, 1, 10, 58, 106, 154]

# Gate order in reference (PyTorch): [i, f, g, o].  Our PSUM layout is
# [i, f, o, g] so sigmoid covers a contiguous [i,f,o] block.
GATE_SRC = [0, 1, 3, 2]
D1_FILL = 12   # PE filler mms after rc (real-work duty alone lets HAM throttle)
F32 = mybir.dt.float32

# --- custom DVE op: out = Src0*(Src1*s0 + s1) ------------------------------
# Fuses the LSTM's tg = 2*sigma(2g)-1 fixup into the i*g product (one DVE
# instruction instead of tensor_scalar + tensor_tensor on the critical
# sigmoid->c->tanh chain).  Registered at import into dve_ops' tables; the
# per-NEFF DVE table mechanism ships the uop program, no firmware change.
from concourse.dve_spec import Spec as _Spec, Src0 as _Src0, Src1 as _Src1
from concourse.dve_spec import C0 as _C0, C1 as _C1, lower as _dve_lower
from concourse.dve_uop import DveOpSpec as _DveOpSpec
from concourse import dve_ops as _DO


def _register_tgii():
    if "LSTM_TGII" in _DO._SUB_OPCODE_FOR_NAME:
        return next(o for o in _DO.OPS if o.name == "LSTM_TGII")
    op = _DO.DveOp(
        "LSTM_TGII", _Spec(body=_Src0 * (_Src1 * _C0 + _C1)),
        subdim=False, uops_sha={},
    )
    _DO.OPS.append(op)
    opcode = 1 + len(_DO.OPS) - 1
    _DO._SUB_OPCODE_FOR_NAME["LSTM_TGII"] = opcode
    _DO.CUSTOM_DVE_SPECS["LSTM_TGII"] = op.spec
    for ver in ("v3", "v4"):
        try:
            _DO._COMPILE_CACHE[("LSTM_TGII", ver)] = _DveOpSpec(
                name="LSTM_TGII", opcode=opcode,
                uops=_dve_lower(op.spec, ver=ver), rd1_en=True,
            )
        except Exception:
            pass
    return op


TGII = _register_tgii()
BF = mybir.dt.bfloat16
AF = mybir.ActivationFunctionType
ALU = mybir.AluOpType


def _pack_layout(use_k64):
    """Column offsets of each constant inside the packed bf16 blob."""
    wih_cols = 2 * H if use_k64 else 4 * H
    o = {}
    o["wih"] = 0
    o["whh"] = o["wih"] + wih_cols
    o["xq"] = o["whh"] + 4 * H
    o["wk"] = o["xq"] + BC
    o["knt"] = o["wk"] + DK
    o["v1"] = o["knt"] + NSLOTS
    o["v2"] = o["v1"] + DV
    o["woh"] = o["v2"] + DV
    o["wocb"] = o["woh"] + DOUT
    o["_end"] = o["wocb"] + DOUT
    return o


def build_kernel(s_steps=S, no_rc=False, use_k64=True):
    nc = bacc.Bacc()

    XP = 2 * DIN if use_k64 else DIN + 1  # k64: x duplicated in both row bands
    WIH_ROWS = 2 * DIN if use_k64 else DIN + 1
    NPAIR = 2 if use_k64 else 4
    O = _pack_layout(use_k64)
    CB = O["_end"]

    xaug = nc.dram_tensor("xaug", [XP, S, BC], BF, kind="ExternalInput")
    cpak = nc.dram_tensor("cpak", [128, CB], BF, kind="ExternalInput")
    fpak = nc.dram_tensor("fpak", [128, 2], F32, kind="ExternalInput")
    out = nc.dram_tensor("out", [DOUT, BC], F32, kind="ExternalOutput")

    with tile.TileContext(nc) as tc, ExitStack() as ctx:
        consts = ctx.enter_context(tc.tile_pool(name="consts", bufs=1))
        xpool = ctx.enter_context(tc.tile_pool(name="xp", bufs=2))
        work = ctx.enter_context(tc.tile_pool(name="work", bufs=3))

        # ---- ALL DMAs ride the sync/HWDGE queue in priority order: the
        # two packed const blobs (needed by everything) FIRST, then the x
        # chunks.  Sequential per-queue transfer order = explicit priority;
        # nothing starves behind the 26MB x stream.
        sb_c = consts.tile([128, CB], BF, tag="cpak")
        nc.sync.dma_start(sb_c[:], cpak[:])
        sb_f = consts.tile([128, 2], F32, tag="fpak")
        nc.sync.dma_start(sb_f[:], fpak[:])
        xc_tiles = {}
        for ci, tc_len in enumerate(CHUNKS):
            t0c = CHUNK_START[ci]
            xc = xpool.tile([XP, max(CHUNKS), BC], BF, tag="xc", name=f"xc{ci}")
            nc.sync.dma_start(xc[:, :tc_len], xaug[:, t0c : t0c + tc_len, :])
            xc_tiles[ci] = xc

        def c_sl(rows, off, width):
            return sb_c[0:rows, off : off + width]

        # ---- small memset consts (DVE, ~100ns each, t~0)
        ones_q = consts.tile([DK, DK], BF, tag="ones_q")
        nc.vector.memset(ones_q[:], 1.0)
        ones_s1 = consts.tile([128, DK], BF, tag="ones_s1")
        nc.vector.memset(ones_s1[:], 1.0)
        ones_s2 = consts.tile([NSLOTS - 128, DK], BF, tag="ones_s2")
        nc.vector.memset(ones_s2[:], 1.0)
        ctxb1 = consts.tile([DV + 1, BC], BF, tag="ctxb1")
        nc.vector.memset(ctxb1[DV : DV + 1, :], 1.0)
        # pre-warm the custom-DVE uop path at FULL op shape: the first
        # 256-col custom op otherwise pays ~4.9us on step 0's chain.
        tgw_in = consts.tile([128, BS], BF, tag="tgw_in")
        nc.vector.memset(tgw_in[:], 0.5)
        tg_warm = consts.tile([128, BS], BF, tag="tg_warm")
        nc.vector._custom_dve(
            TGII, out=tg_warm[:], in0=tgw_in[:], in1=tgw_in[:],
            s0=2.0, s1=-1.0,
        )
        # pre-warm the sigmoid ACT table: the auto-inserted table load
        # sits BEHIND sigma_0's semaphore wait on the ACT queue, so
        # without this it only starts once step 0's matmuls finish.
        sig_warm = consts.tile([128, 8], BF, tag="sig_warm")
        nc.scalar.activation(sig_warm[:], ones_s1[:, 0:8], AF.Sigmoid)

        # (no PE warm-up burst: the scan's own filler duty ramps the HAM
        # clock within the first couple of steps; a pre-scan burst either
        # blocks a psum bank or delays step 0 more than the ramp costs)

        # ---- attention q / ||q||^2: pre-scan, one psum bank, results
        # copied straight to SBUF so the bank frees before the scan.
        # All bf16 (the downstream logits matmul is bf16 anyway).
        # NOTE: no Pool-engine compute anywhere -- Q7 tensor ops were
        # observed to stall concurrent custom-DVE ops by multiple us.
        psA_cm = tc.tile_pool(name="psA", bufs=1, space="PSUM")
        psA = psA_cm.__enter__()
        q_ps = psA.tile([DK, BC], F32, tag="q")
        nc.tensor.matmul(
            q_ps[:], c_sl(KD + 1, O["wk"], DK), c_sl(KD + 1, O["xq"], BC),
            start=True, stop=True,
        )
        # q2 via ACT Square straight from psum (ACT is idle pre-scan and
        # Square is in the sigmoid table set), in PARALLEL with the DVE
        # qc copy -- shortens the serial chain that gates sigma_0 via
        # the psum-bank WAR.
        qc = consts.tile([DK, BC], BF, tag="qc")
        nc.vector.tensor_copy(qc[:], q_ps[:])
        q2 = consts.tile([DK, BC], BF, tag="q2")
        nc.scalar.activation(q2[:], q_ps[:], AF.Square)
        nc.tensor.matmul(q_ps[:], ones_q[:], q2[:], start=True, stop=True)
        nsb = consts.tile([DK, BC], BF, tag="nsb")
        nc.vector.tensor_copy(nsb[:], q_ps[:])
        psA_cm.__exit__(None, None, None)

        def emit_newton():
            # rsqrt(n) Newton on the DVE, all-bf16 (2x/4x modes): clamped
            # linear seed + 2 iters y <- y*(1.5 - 0.5*n*y^2).  Emitted
            # after step 0's DVE ops: the DVE is idle there while the
            # ACT/PE ramp, so the ~4us chain hides in the early bubbles.
            y0 = consts.tile([DK, BC], BF, tag="y0")
            nc.vector.tensor_scalar(y0[:], nsb[:], -0.002953, 0.2917,
                                    ALU.mult, ALU.add)
            yc = consts.tile([DK, BC], BF, tag="yc")
            nc.vector.tensor_scalar(yc[:], y0[:], 0.085, 0.40, ALU.max, ALU.min)
            s32 = yc
            for it in range(2):
                t2 = consts.tile([DK, BC], BF, tag=f"nw_t{it}")
                nc.vector.tensor_tensor(t2[:], s32[:], s32[:], ALU.mult)
                u2 = consts.tile([DK, BC], BF, tag=f"nw_u{it}")
                nc.vector.tensor_tensor(u2[:], t2[:], nsb[:], ALU.mult)
                v2 = consts.tile([DK, BC], BF, tag=f"nw_v{it}")
                nc.vector.tensor_scalar(v2[:], u2[:], -0.5, 1.5,
                                        ALU.mult, ALU.add)
                y2 = consts.tile([DK, BC], BF, tag=f"nw_y{it}")
                nc.vector.tensor_tensor(y2[:], s32[:], v2[:], ALU.mult)
                s32 = y2
            qn = consts.tile([DK, BC], BF, tag="qn")
            nc.vector.tensor_tensor(qn[:], qc[:], s32[:], ALU.mult)
            return qn

        # ========================= LSTM scan ===========================
        psL_cm = tc.tile_pool(name="psL", bufs=1, space="PSUM")
        psL = psL_cm.__enter__()
        h_tiles = [None, None]
        c_tiles = [None, None]

        _dummy_rhs = sb_c[:, O["whh"] + H : O["whh"] + 3 * H]
        _dummy_lhs = sb_c[:, O["whh"] + H : O["whh"] + H + 32]
        # psum per stream: [128, 4, 512] fp32, ONE BANK PER GATE (cols
        # 0:BS used).  start=True clears has_written at bank granularity,
        # so bank isolation lets ic mms prefetch in any order before the
        # h-dependent rc mms.  bufs=1: 4 banks x 2 streams = all of PSUM;
        # the only psum reader is the merged sigmoid, so step t+1's ic
        # mms just wait on sigma(t).
        chunk_of = []
        for ci, tc_len in enumerate(CHUNKS):
            chunk_of += [(ci, t) for t in range(tc_len)]
        for t in range(s_steps):
            xc = xc_tiles[chunk_of[t][0]]
            ti = chunk_of[t][1]
            ps_t = []
            for s in range(NSTREAM):
                ps = psL.tile([128, 4, 512], F32, tag=f"lstm{s}")
                ps_t.append(ps)
            # input-projection mms: prefetchable (no h dependency).
            # k64: two gates run CONCURRENTLY in the PE array via row
            # tiling (bands at rows 0-63 / 64-127); x is duplicated in
            # both partition bands so each band streams its own copy.
            for p in range(NPAIR):
                for s in range(NSTREAM):
                    if use_k64:
                        for b in range(2):
                            g = 2 * p + b
                            lh = sb_c[
                                64 * b : 64 * (b + 1),
                                O["wih"] + p * H : O["wih"] + (p + 1) * H,
                            ]
                            rh = xc[64 * b : 64 * (b + 1), ti,
                                    s * BS : (s + 1) * BS]
                            nc.tensor.matmul(
                                ps_t[s][:, g, 0:BS], lh, rh,
                                start=True, stop=(t == 0),
                                tile_position=(64 * b, 0),
                            )
                    else:
                        xs = xc[:, ti, s * BS : (s + 1) * BS]
                        lh = sb_c[
                            0:WIH_ROWS,
                            O["wih"] + p * H : O["wih"] + (p + 1) * H,
                        ]
                        nc.tensor.matmul(
                            ps_t[s][:, p, 0:BS], lh, xs,
                            start=True, stop=(t == 0),
                        )

            # PE filler: keep duty high so HAM holds the 2.4GHz clock.
            # 32-col weights make the LDWEIGHTS ~27ns (vs 96 for 128-col)
            # and the long 256-col stream supplies the array duty; dummies
            # write the unused scratch half of the step's psum banks.
            def dummy(n, tile_idx=0, gate=0):
                for _ in range(n):
                    nc.tensor.matmul(
                        ps_t[tile_idx][0:32, gate, BS : BS + 256],
                        _dummy_lhs,
                        _dummy_rhs[:, 0:256], start=False, stop=False,
                        skip_group_check=True,
                    )
            if t == 0:
                # HAM p-state ramp burst right after step-0's input
                # projections.  Target = a stream-1 gate bank that does
                # NOT overlap the attention q bank (stream 0, gate 0), so
                # sigma_A(0) doesn't wait on it; kept short so sigma_B(0)
                # stays ~1.1us behind sigma_A(0) -- the steady-state
                # stream phase locks in from these initial conditions.
                dummy(12, tile_idx=1, gate=2)
            # recurrent mms
            if t > 0 and not no_rc:
                for s in range(NSTREAM):
                    for g in range(4):
                        nc.tensor.matmul(
                            ps_t[s][:, g, 0:BS],
                            sb_c[:, O["whh"] + g * H : O["whh"] + (g + 1) * H],
                            h_tiles[s][:],
                            start=False, stop=True,
                        )

            # merged sigmoid over all 4 gates (g gate pre-scaled by 2 so
            # tanh(x) = 2*sigmoid(2x)-1 costs only a DVE fixup)
            sigs = []
            for s in range(NSTREAM):
                sig = work.tile([128, 4, BS], BF, tag=f"sig{s}")
                nc.scalar.activation(sig[:], ps_t[s][:, :, 0:BS], AF.Sigmoid)
                sigs.append(sig)
            if t > 0:
                # PE warmth filler AFTER sigma_A's emission: the bank-order
                # dep (scratch shares stream A's banks) is then on the
                # already-completed sigma_A, so the fillers run in the PE's
                # idle window without delaying sigma, ic or rc.
                dummy(D1_FILL, 0)
            iis, ffs = [], []
            for s in range(NSTREAM):
                sig = sigs[s]
                # fused ii = sigma_i * (2*sigma(2g) - 1) -- one DVE instr on
                # the critical chain instead of tensor_scalar+tensor_tensor
                ii = work.tile([128, BS], BF, tag=f"ii{s}")
                nc.vector._custom_dve(
                    TGII, out=ii[:], in0=sig[:, 0], in1=sig[:, 3],
                    s0=2.0, s1=-1.0,
                )
                iis.append(ii)
                if t > 0:
                    ff = work.tile([128, BS], BF, tag=f"ff{s}")
                    nc.vector.tensor_tensor(ff[:], sig[:, 1], c_tiles[s][:], ALU.mult)
                    ffs.append(ff)
            c_news = []
            for s in range(NSTREAM):
                c_new = work.tile([128, BS], BF, tag=f"c{s}")
                if t > 0:
                    nc.vector.tensor_tensor(c_new[:], iis[s][:], ffs[s][:], ALU.add)
                else:
                    nc.vector.tensor_copy(c_new[:], iis[s][:])
                c_news.append(c_new)
            c_tiles = c_news
            h_news = []
            for s in range(NSTREAM):
                tcc = work.tile([128, BS], BF, tag=f"tc{s}")
                nc.scalar.activation(tcc[:], c_tiles[s][:], AF.Tanh)
                h_new = work.tile([128, BS], BF, tag=f"h{s}")
                nc.vector.tensor_tensor(h_new[:], sigs[s][:, 2], tcc[:], ALU.mult)
                h_news.append(h_new)
            h_tiles = h_news
            if t == 0:
                qn = emit_newton()

        psL_cm.__exit__(None, None, None)

        # ================ attention read + output head (tail) ==========
        # Emitted after the scan on every engine queue: the exp table
        # load (which evicts the sigmoid set) lands here, and l/exp/
        # softmax deps (qn from the Pool Newton) are long since ready.
        psH_cm = tc.tile_pool(name="psH", bufs=1, space="PSUM")
        psH = psH_cm.__enter__()
        # one psum bank PER STREAM: a second start=True in a shared bank
        # would clear the first stream's has_written state (bank-granular)
        out_ps = [
            psH.tile([DOUT, BS], F32, tag=f"o{s}", name=f"out_ps{s}")
            for s in range(NSTREAM)
        ]
        for s in range(NSTREAM):
            nc.tensor.matmul(
                out_ps[s][:], c_sl(H, O["woh"], DOUT), h_tiles[s][:],
                start=True, stop=False,
            )
        # logits chunks: L = kn @ qn  ([slots, BC]); bf16 -> 1 cyc/row on PE
        l1_ps = psH.tile([128, BC], F32, tag="l1")
        nc.tensor.matmul(
            l1_ps[:], c_sl(DK, O["knt"], 128), qn[:], start=True, stop=True
        )
        l2_ps = psH.tile([NSLOTS - 128, BC], F32, tag="l2")
        nc.tensor.matmul(
            l2_ps[:], sb_c[0:DK, O["knt"] + 128 : O["knt"] + NSLOTS], qn[:],
            start=True, stop=True,
        )
        # softmax without max-subtraction: masked slots get bias -1e5 ->
        # exp underflows to 0.  bias_ptr = per-partition mask column.
        e1 = consts.tile([128, BC], BF, tag="e1")
        nc.scalar.activation(e1[:], l1_ps[:], AF.Exp, bias=sb_f[0:128, 0:1],
                             scale=1.0 / TEMP)
        e2 = consts.tile([NSLOTS - 128, BC], BF, tag="e2")
        nc.scalar.activation(e2[:], l2_ps[:], AF.Exp,
                             bias=sb_f[0 : NSLOTS - 128, 1:2],
                             scale=1.0 / TEMP)
        # denominator, replicated over DK partitions; rs = 1/S
        s_ps = psH.tile([DK, BC], F32, tag="s")
        nc.tensor.matmul(s_ps[:], ones_s1[:], e1[:], start=True, stop=False)
        nc.tensor.matmul(s_ps[:], ones_s2[:], e2[:], start=False, stop=True)
        rs = consts.tile([DK, BC], F32, tag="rs")
        nc.vector.reciprocal_approx_fast(rs[:], s_ps[:])
        # ctxT = V^T @ E -> [DV, BC]; normalize into ctxb1 rows 0:DV
        # (row DV is the ones-row that carries b_o through the matmul)
        cx_ps = psH.tile([DV, BC], F32, tag="c")
        nc.tensor.matmul(cx_ps[:], c_sl(128, O["v1"], DV), e1[:],
                         start=True, stop=False)
        nc.tensor.matmul(cx_ps[:], sb_c[0 : NSLOTS - 128, O["v2"] : O["v2"] + DV],
                         e2[:], start=False, stop=True)
        nc.vector.tensor_tensor(ctxb1[0:DV, :], cx_ps[:], rs[:], ALU.mult)
        for s in range(NSTREAM):
            cols = slice(s * BS, (s + 1) * BS)
            nc.tensor.matmul(
                out_ps[s][:], c_sl(DV + 1, O["wocb"], DOUT),
                ctxb1[:, cols], start=False, stop=True,
            )
        # per-stream copy + DMA pairs so stream B's copy overlaps stream
        # A's DMA on the tail
        out_sb = consts.tile([DOUT, BC], F32, tag="out_sb")
        for s in range(NSTREAM):
            cols = slice(s * BS, (s + 1) * BS)
            nc.vector.tensor_copy(out_sb[:, cols], out_ps[s][:])
            nc.sync.dma_start(out[:, cols], out_sb[:, cols])
        psH_cm.__exit__(None, None, None)

    nc.finalize()
    return nc


def _prep_inputs(inputs, W_ih, W_hh, b_ih, b_hh, W_k, b_k, W_o, b_o):
    """Host-side prep: weight layouts, memory bank, per-core shards."""
    f32 = np.float32
    inputs = np.asarray(inputs, f32)
    W_ih = np.asarray(W_ih, f32)
    W_hh = np.asarray(W_hh, f32)
    b = np.asarray(b_ih, f32) + np.asarray(b_hh, f32)
    W_k = np.asarray(W_k, f32)
    b_k = np.asarray(b_k, f32)
    W_o = np.asarray(W_o, f32)
    b_o = np.asarray(b_o, f32)

    use_k64 = bool(np.all(b == 0.0))
    O = _pack_layout(use_k64)
    CB = O["_end"]

    # LSTM weights, gate-transposed, gate order [i,f,o,g]; tanh(x) =
    # 2*sigmoid(2x)-1 folded into the g gate (position 3).
    whh = np.zeros((H, 4, H), f32)
    for j, gs in enumerate(GATE_SRC):
        rows = slice(gs * H, (gs + 1) * H)
        whh[:, j, :] = W_hh[rows].T
    whh[:, 3, :] *= 2.0
    if use_k64:
        # row-tiled pairs: pair p holds gates (2p, 2p+1) in partition
        # bands 0:64 / 64:128
        wih = np.zeros((2 * DIN, 2, H), f32)
        for j, gs in enumerate(GATE_SRC):
            rows = slice(gs * H, (gs + 1) * H)
            band, pair = (j % 2) * DIN, j // 2
            wih[band : band + DIN, pair, :] = W_ih[rows].T
            if j == 3:
                wih[band : band + DIN, pair, :] *= 2.0
        wih_rows, wih_cols = 2 * DIN, 2 * H
    else:
        wih = np.zeros((DIN + 1, 4, H), f32)
        for j, gs in enumerate(GATE_SRC):
            rows = slice(gs * H, (gs + 1) * H)
            wih[:DIN, j, :] = W_ih[rows].T
            wih[DIN, j, :] = b[rows]
        wih[:, 3, :] *= 2.0
        wih_rows, wih_cols = DIN + 1, 4 * H

    # memory bank from sample 0 (host-side, replicated)
    support = inputs[0, :NSLOTS]
    kp, vp = support[:, :KD], support[:, KD:]
    active = vp.sum(axis=-1) >= NOVELTY
    sk = kp @ W_k.T + b_k
    kn = sk / (np.linalg.norm(sk, axis=-1, keepdims=True) + EPS)
    knt = np.ascontiguousarray(kn.T)  # [DK, NSLOTS]
    maskb = np.where(active, 0.0, -1e5).astype(f32)

    wk = np.zeros((KD + 1, DK), f32)
    wk[:KD] = W_k.T
    wk[KD] = b_k

    woh = np.ascontiguousarray(W_o[:, :H].T)        # [H, DOUT]
    wocb = np.zeros((DV + 1, DOUT), f32)            # [woc ; b_o]
    wocb[:DV] = W_o[:, H:].T
    wocb[DV] = b_o

    # ---- packed const blobs
    cpak = np.zeros((128, CB), f32)
    cpak[0:wih_rows, O["wih"] : O["wih"] + wih_cols] = wih.reshape(
        wih_rows, wih_cols
    )
    cpak[0:H, O["whh"] : O["whh"] + 4 * H] = whh.reshape(H, 4 * H)
    cpak[0 : KD + 1, O["wk"] : O["wk"] + DK] = wk
    cpak[0:DK, O["knt"] : O["knt"] + NSLOTS] = knt
    cpak[0:128, O["v1"] : O["v1"] + DV] = vp[0:128]
    cpak[0 : NSLOTS - 128, O["v2"] : O["v2"] + DV] = vp[128:NSLOTS]
    cpak[0:H, O["woh"] : O["woh"] + DOUT] = woh
    cpak[0 : DV + 1, O["wocb"] : O["wocb"] + DOUT] = wocb

    fpak = np.zeros((128, 2), f32)
    fpak[0:128, 0] = maskb[0:128]
    fpak[0 : NSLOTS - 128, 1] = maskb[128:NSLOTS]

    in_maps = []
    for c in range(NCORES):
        shard = inputs[c * BC : (c + 1) * BC]  # [BC, S, DIN]
        xt = shard.transpose(2, 1, 0)  # [DIN, S, BC]
        if use_k64:
            xaug = np.empty((2 * DIN, S, BC), f32)
            xaug[:DIN] = xt
            xaug[DIN:] = xt
        else:
            xaug = np.empty((DIN + 1, S, BC), f32)
            xaug[:DIN] = xt
            xaug[DIN] = 1.0
        cp = cpak.copy()
        cp[0 : KD + 1, O["xq"] : O["xq"] + BC] = np.concatenate(
            [shard[:, S - 1, :KD].T, np.ones((1, BC), f32)], axis=0
        )
        m = dict(
            xaug=xaug.astype(BF16),
            cpak=cp.astype(BF16),
            fpak=fpak,
        )
        in_maps.append(m)
    return in_maps


_CACHED_NC = None


def kernel(inputs, W_ih, W_hh, b_ih, b_hh, W_k, b_k, W_o, b_o,
           _trace=False, _return_raw=False):
    global _CACHED_NC
    in_maps = _prep_inputs(inputs, W_ih, W_hh, b_ih, b_hh, W_k, b_k, W_o, b_o)
    use_k64 = in_maps[0]["xaug"].shape[0] == 2 * DIN
    if _CACHED_NC is None:
        _CACHED_NC = build_kernel(use_k64=use_k64)
    res = run_bass_kernel_spmd(
        _CACHED_NC, in_maps, core_ids=list(range(NCORES)), trace=_trace
    )
    outs = [np.asarray(res.results[i]["out"], np.float32).T for i in range(NCORES)]
    full = np.concatenate(outs, axis=0)
    if _return_raw:
        return full, res
    return full
